# revision 13
# baseline (speedup 1.0000x reference)
"""MoE FFN (top-2 of 8 experts, SwiGLU) for 8 Trainium2 NeuronCores.

Strategy: expert parallelism with load-balanced expert PAIRING. The router
(tiny [T,H]@[H,E] matmul + softmax + top-2) runs on host as part of sharding.
Experts are sorted by routed-token count and paired big-with-small; each pair
is served by two cores, and BOTH experts' token lists are split between those
two cores. Every core therefore runs the same program shape: segment A
(s1 tokens of the pair's heavy expert) + segment B (s2 tokens of the light
expert), with s1 = ceil(max_heavy/2), s2 = ceil(max_light/2). This cuts the
padded per-core token count from max_e(load_e) (~2184) to
(max_heavy + max_light)/2 (~2112), i.e. every core does near-average work.

Each core runs a dense SwiGLU FFN over its tokens in bf16 (fp32 PSUM), in a
feature-on-partition / token-on-free-dim layout so no on-device transposes
are needed and every weight byte is DMA'd exactly once per segment, as large
contiguous transfers. The host applies combine weights and scatter-adds.

Per-core device program per segment (expert e of that segment), with
nht = H/128 h-tiles, f-chunks of FCH columns (NFT f-tiles each):
  g_T[f, t] = sum_i w1[h_i, f]^T @ x_T[h_i, t]        (PSUM accum over h-tiles)
  u_T[f, t] likewise with w2
  h_T[f, t] = silu(g_T + b1) * (u_T + b2)             (ACT + DVE, -> bf16)
  y_T[h, t] = sum_f w3[f, h]^T @ h_T[f, t] + b3       (PSUM accum per f-chunk,
                                                       accumulated in SBUF f32)
Weights for both segments stream through SBUF one f-chunk at a time
(double-buffered per (tensor, segment) tag); tokens/outputs are SBUF-resident.
Every matmul has a 128-row stationary operand in natural layout and a
[128, block>=128] moving operand, so the PE runs back-to-back at stream rate.
The first block is small (<=128) so the PE starts as soon as ~0.5 MB of DMA
lands; the last block is small so the final y write-back tail is short.
"""

import numpy as np
import ml_dtypes

E = 8       # experts
K = 2       # top-k
H = 1024    # hidden
F = 4096    # ffn dim
FCH = 256   # f-chunk size (weight streaming granularity); FCH % 128 == 0

NHT = H // 128    # h-tiles
NFCH = F // FCH   # f-chunks
NFT = FCH // 128  # f-tiles per chunk

_BF16 = ml_dtypes.bfloat16

_kernel_cache: dict[object, object] = {}
_last_in_maps = None


def _split_sizes(n: int, first_small: bool):
    """Split n (multiple of 8) into as few blocks as possible, each 256..512
    cols (a single smaller block only if n < 256): fewer blocks mean fewer
    matmul instructions per column, and >=256 moving columns keep the PE at
    stream rate (the 128-cycle stationary load stays hidden)."""
    sizes = []
    while n > 1024:
        sizes.append(512)
        n -= 512
    if n > 512:
        a = (n // 2 + 7) // 8 * 8
        sizes.extend([a, n - a])
    elif n:
        sizes.append(n)
    sizes.sort(reverse=True)
    if first_small:
        sizes = sizes[::-1]
    return sizes


def _blocks_of(sizes):
    blocks, off = [], 0
    for sz in sizes:
        blocks.append((off, sz))
        off += sz
    return blocks, off


def _build(blocksA, blocksB, use_b2: bool):
    """Build the per-core Bass/Tile program for the given segment blocks."""
    import concourse.bass as bass  # noqa: F401
    import concourse.tile as tile
    from concourse import bacc, mybir

    bf16 = mybir.dt.bfloat16
    f32 = mybir.dt.float32
    AF = mybir.ActivationFunctionType

    capA = sum(sz for _, sz in blocksA)
    capB = sum(sz for _, sz in blocksB)
    caps = capA + capB

    nc = bacc.Bacc("TRN2", target_bir_lowering=False, debug=False, num_devices=E)

    xT = nc.declare_dram_parameter("xT", [128, NHT * caps], bf16, isOutput=False)
    wd = {}
    for s in "ab":
        for t in ("w1", "w2", "w3"):
            wd[t + s] = nc.declare_dram_parameter(
                t + s, [NFCH, 128, NFT * H], bf16, isOutput=False
            )
    bd = {}
    for s in "ab":
        bd["b1" + s] = nc.declare_dram_parameter(
            "b1" + s, [128, F // 128], f32, isOutput=False
        )
        bd["b3" + s] = nc.declare_dram_parameter(
            "b3" + s, [128, NHT], f32, isOutput=False
        )
        if use_b2:
            bd["b2" + s] = nc.declare_dram_parameter(
                "b2" + s, [128, F // 128], f32, isOutput=False
            )
    yT = nc.declare_dram_parameter("yT", [128, NHT * caps], f32, isOutput=True)

    with tile.TileContext(nc) as tc:
        with (
            tc.tile_pool(name="xp", bufs=1) as xp,
            tc.tile_pool(name="yp", bufs=1) as yp,
            tc.tile_pool(name="wp", bufs=2) as wp,
            tc.tile_pool(name="hp", bufs=2) as hp,
            tc.tile_pool(name="sp", bufs=2) as sp,
            tc.tile_pool(name="bp", bufs=1) as bp,
            tc.tile_pool(name="pg", bufs=2, space="PSUM") as pg,
            tc.tile_pool(name="pu", bufs=2, space="PSUM") as pu,
            tc.tile_pool(name="py", bufs=3, space="PSUM") as py,
        ):
            # Tokens (resident, bf16), block-major column order: block at
            # global token offset `goff` spans cols [NHT*goff, NHT*(goff+sz)),
            # h-tile i contiguous inside it. The host supplies the identical
            # layout, so each block is ONE contiguous 2D transfer.
            xall = xp.tile([128, NHT * caps], bf16, name="xall")

            # Output accumulator (resident, f32), i-major columns.
            yall = yp.tile([128, NHT * caps], f32, name="yall")

            segs = (("a", blocksA, 0), ("b", blocksB, capA))

            # Prologue: first block's x (small) + first f-chunk's w1 for
            # segment a lead both queues so the first matmul waits only on
            # ~1MB of DMA; then everything else streams behind the PE.
            offA0, szA0 = blocksA[0]
            nc.gpsimd.dma_start(xall[:, 0:NHT * szA0], xT[:, 0:NHT * szA0])
            wtiles = {}
            bt = {}
            # Segment a's first w1/w2 chunks lead their queues (the PE needs
            # them immediately); all x blocks follow on the scalar queue
            # BEFORE segment b's first chunks (needed ~30us later).
            w1c = wp.tile([128, NFT * H], bf16, tag="w1a", name="w1c")
            w2c = wp.tile([128, NFT * H], bf16, tag="w2a", name="w2c")
            for j in range(NFT):
                jsl = slice(j * H, (j + 1) * H)
                nc.sync.dma_start(w1c[:, jsl], wd["w1a"][0][:, jsl])
                nc.scalar.dma_start(w2c[:, jsl], wd["w2a"][0][:, jsl])
            b1t = bp.tile([128, F // 128], f32, tag="b1a", name="b1t")
            nc.sync.dma_start(b1t[:], bd["b1a"][:])
            bt["b1a"] = b1t
            w3c = wp.tile([128, NFT * H], bf16, tag="w3a", name="w3c")
            nc.sync.dma_start(w3c[:], wd["w3a"][0])
            wtiles["a"] = (w1c, w2c, w3c)
            # Remaining token blocks, in consumption order.
            first = True
            for s, blocks, base in segs:
                for off, sz in blocks:
                    if first:
                        first = False
                        continue
                    lo, hi = NHT * (base + off), NHT * (base + off + sz)
                    nc.gpsimd.dma_start(xall[:, lo:hi], xT[:, lo:hi])
            # Remaining biases (tiny, resident), then segment b's chunks.
            for s in "ab":
                if s != "a":
                    b1t = bp.tile([128, F // 128], f32, tag="b1" + s,
                                  name="b1t")
                    nc.sync.dma_start(b1t[:], bd["b1" + s][:])
                    bt["b1" + s] = b1t
                b3t = bp.tile([128, NHT], f32, tag="b3" + s, name="b3t")
                nc.sync.dma_start(b3t[:], bd["b3" + s][:])
                bt["b3" + s] = b3t
                if use_b2:
                    b2t = bp.tile([128, F // 128], f32, tag="b2" + s, name="b2t")
                    nc.sync.dma_start(b2t[:], bd["b2" + s][:])
                    bt["b2" + s] = b2t
            w1c = wp.tile([128, NFT * H], bf16, tag="w1b", name="w1c")
            nc.sync.dma_start(w1c[:], wd["w1b"][0])
            w2c = wp.tile([128, NFT * H], bf16, tag="w2b", name="w2c")
            nc.scalar.dma_start(w2c[:], wd["w2b"][0])
            w3c = wp.tile([128, NFT * H], bf16, tag="w3b", name="w3c")
            nc.sync.dma_start(w3c[:], wd["w3b"][0])
            wtiles["b"] = (w1c, w2c, w3c)

            def stage_b(w3c, b3t, goff, sz, ht, fc):
                # y_T[h, tok] += w3_chunk.T @ h_T ; w3c cols: (j, h).
                for i in range(NHT):
                    psy = py.tile([128, sz], f32, tag="y", name="psy")
                    for j in range(NFT):
                        nc.tensor.matmul(
                            psy[:],
                            w3c[:, j * H + i * 128:j * H + (i + 1) * 128],
                            ht[:, j * sz:(j + 1) * sz],
                            start=(j == 0), stop=(j == NFT - 1),
                        )
                    lo = i * caps + goff
                    dst = yall[:, lo:lo + sz]
                    if fc == 0:
                        nc.scalar.activation(
                            dst, psy[:], AF.Identity, bias=b3t[:, i:i + 1]
                        )
                    else:
                        nc.vector.tensor_add(dst, dst, psy[:])
                    if fc == NFCH - 1:
                        nc.sync.dma_start(yT[:, lo:lo + sz], dst)

            pending = None  # (w3c, b3t, goff, sz, ht, fc) awaiting stage B
            for fc in range(NFCH):
                for s, blocks, base in segs:
                    if fc == 0:
                        w1c, w2c, w3c = wtiles[s]
                    else:
                        # Stream this f-chunk's weights (each byte once).
                        w1c = wp.tile([128, NFT * H], bf16, tag="w1" + s,
                                      name="w1c")
                        nc.sync.dma_start(w1c[:], wd["w1" + s][fc])
                        w2c = wp.tile([128, NFT * H], bf16, tag="w2" + s,
                                      name="w2c")
                        nc.scalar.dma_start(w2c[:], wd["w2" + s][fc])
                        w3c = wp.tile([128, NFT * H], bf16, tag="w3" + s,
                                      name="w3c")
                        nc.sync.dma_start(w3c[:], wd["w3" + s][fc])
                    b1t = bt["b1" + s]
                    for off, sz in blocks:
                        goff = base + off
                        xb = NHT * goff
                        # Stage A: h_T = silu(g_T + b1) * (u_T + b2)
                        # w1c/w2c cols: (j, i, q).
                        ht = hp.tile([128, NFT * sz], bf16, tag="h", name="ht",
                                     padded_shape=[128, NFT * 512])
                        for j in range(NFT):
                            fg = fc * NFT + j
                            psg = pg.tile([128, sz], f32, tag="g", name="psg",
                                          padded_shape=[128, 512])
                            for i in range(NHT):
                                wb = (j * NHT + i) * 128
                                nc.tensor.matmul(
                                    psg[:], w1c[:, wb:wb + 128],
                                    xall[:, xb + i * sz:xb + (i + 1) * sz],
                                    start=(i == 0), stop=(i == NHT - 1),
                                )
                            stile = sp.tile([128, sz], f32, tag="s",
                                            name="stile", padded_shape=[128, 512])
                            nc.scalar.activation(
                                stile[:], psg[:], AF.Silu,
                                bias=b1t[:, fg:fg + 1],
                            )
                            psu = pu.tile([128, sz], f32, tag="u", name="psu",
                                          padded_shape=[128, 512])
                            for i in range(NHT):
                                wb = (j * NHT + i) * 128
                                nc.tensor.matmul(
                                    psu[:], w2c[:, wb:wb + 128],
                                    xall[:, xb + i * sz:xb + (i + 1) * sz],
                                    start=(i == 0), stop=(i == NHT - 1),
                                )
                            hsl = ht[:, j * sz:(j + 1) * sz]
                            if use_b2:
                                u2 = sp.tile([128, sz], f32, tag="u2",
                                             name="u2tile",
                                             padded_shape=[128, 512])
                                nc.scalar.activation(
                                    u2[:], psu[:], AF.Identity,
                                    bias=bt["b2" + s][:, fg:fg + 1],
                                )
                                nc.vector.tensor_mul(hsl, stile[:], u2[:])
                            else:
                                nc.vector.tensor_mul(hsl, stile[:], psu[:])

                        if pending is not None:
                            stage_b(*pending)
                        pending = (w3c, bt["b3" + s], goff, sz, ht, fc)
            stage_b(*pending)

    nc.finalize()
    return nc


def _route(x2d: np.ndarray, router_w: np.ndarray):
    """Host router: softmax over experts, top-2. Returns per-expert token
    index lists and combine weights."""
    logits = x2d @ router_w                       # [T, E]
    logits -= logits.max(axis=-1, keepdims=True)
    p = np.exp(logits, dtype=np.float32)
    p /= p.sum(axis=-1, keepdims=True)
    order = np.argsort(-p, axis=-1, kind="stable")[:, :K]   # [T, K]
    idx_e, cw_e = [], []
    for e in range(E):
        sel = np.nonzero((order == e).any(axis=1))[0]
        idx_e.append(sel)
        cw_e.append(p[sel, e])
    return idx_e, cw_e


def _pack_w12(w: np.ndarray) -> np.ndarray:
    """[H, F] f32 -> [NFCH, 128, NFT*H] bf16 with column order (j, i, q):
    chunk c, partition p, f-tile j, h-tile i, col q = w[i*128+p, c*FCH+j*128+q].
    """
    t = np.asarray(w, dtype=np.float32).reshape(NHT, 128, NFCH, NFT, 128)
    t = t.transpose(2, 1, 3, 0, 4)  # [c, p, j, i, q]
    return np.ascontiguousarray(t.astype(_BF16)).reshape(NFCH, 128, NFT * H)


def _pack_w3(w: np.ndarray) -> np.ndarray:
    """[F, H] f32 -> [NFCH, 128, NFT*H] bf16 with column order (j, h):
    chunk c, partition p (= f within f-tile j) -> w[c*FCH+j*128+p, h]."""
    t = np.asarray(w, dtype=np.float32).reshape(NFCH, NFT, 128, H)
    t = t.transpose(0, 2, 1, 3)  # [c, p, j, h]
    return np.ascontiguousarray(t.astype(_BF16)).reshape(NFCH, 128, NFT * H)


def _pack_x(x2d_rows: np.ndarray, blocks, cap) -> np.ndarray:
    """[n, H] f32 rows -> [128, NHT*cap] bf16 in block-major column order."""
    n = len(x2d_rows)
    xg = np.zeros((cap, H), dtype=_BF16)
    xg[:n] = x2d_rows.astype(_BF16)
    return np.concatenate(
        [
            xg[off:off + sz].reshape(sz, NHT, 128)
            .transpose(2, 1, 0).reshape(128, NHT * sz)
            for off, sz in blocks
        ],
        axis=1,
    )


def kernel(x, router_w, w1, b1, w2, b2, w3, b3):
    from concourse.bass_utils import run_bass_kernel_spmd

    B, S, _ = x.shape
    T = B * S
    x2d = np.ascontiguousarray(x, dtype=np.float32).reshape(T, H)

    idx_e, cw_e = _route(x2d, np.asarray(router_w, dtype=np.float32))
    loads = np.array([len(i) for i in idx_e])
    order = np.argsort(-loads, kind="stable")
    bigs, smalls = order[:4], order[4:]
    # s1 covers half the heaviest expert; s2 half the heaviest of the smalls.
    s1 = -(-(int(loads[bigs[0]]) + 1) // 2 // 8) * 8
    s2 = -(-(int(loads[smalls[0]]) + 1) // 2 // 8) * 8
    s1 = max(s1, 16)
    s2 = max(s2, 16)
    blocksA, capA = _blocks_of(_split_sizes(s1, first_small=True))
    blocksB, capB = _blocks_of(_split_sizes(s2, first_small=False))
    caps = capA + capB

    use_b2 = bool(np.any(b2))
    key = (tuple(blocksA), tuple(blocksB), use_b2)
    nc = _kernel_cache.get(key)
    if nc is None:
        nc = _build(blocksA, blocksB, use_b2)
        _kernel_cache[key] = nc

    # Pack per-expert weights once; cores of a pair share the arrays.
    wpk = {}
    for e in range(E):
        wpk[e] = (_pack_w12(w1[e]), _pack_w12(w2[e]), _pack_w3(w3[e]))
    bpk = {}
    for e in range(E):
        bpk[e] = (
            np.ascontiguousarray(
                np.asarray(b1[e], dtype=np.float32).reshape(F // 128, 128).T),
            np.ascontiguousarray(
                np.asarray(b3[e], dtype=np.float32).reshape(NHT, 128).T),
            np.ascontiguousarray(
                np.asarray(b2[e], dtype=np.float32).reshape(F // 128, 128).T)
            if use_b2 else None,
        )

    # Split each expert's tokens between the two cores of its pair.
    seg_tokens = []  # per core: (idxA, cwA, idxB, cwB)
    for pr in range(4):
        eA, eB = int(bigs[pr]), int(smalls[pr])
        for half in range(2):
            ia, ca = idx_e[eA], cw_e[eA]
            ib, cb = idx_e[eB], cw_e[eB]
            na, nb = (len(ia) + 1) // 2, (len(ib) + 1) // 2
            sa = slice(0, na) if half == 0 else slice(na, None)
            sb = slice(0, nb) if half == 0 else slice(nb, None)
            seg_tokens.append((eA, ia[sa], ca[sa], eB, ib[sb], cb[sb]))

    in_maps = []
    for c in range(E):
        eA, ia, ca, eB, ib, cb = seg_tokens[c]
        xTe = np.ascontiguousarray(np.concatenate(
            [_pack_x(x2d[ia], blocksA, capA), _pack_x(x2d[ib], blocksB, capB)],
            axis=1,
        ))
        m = {"xT": xTe}
        for s, e in (("a", eA), ("b", eB)):
            m["w1" + s], m["w2" + s], m["w3" + s] = wpk[e]
            m["b1" + s], m["b3" + s] = bpk[e][0], bpk[e][1]
            if use_b2:
                m["b2" + s] = bpk[e][2]
        in_maps.append(m)

    global _last_in_maps
    _last_in_maps = in_maps
    res = run_bass_kernel_spmd(nc, in_maps, core_ids=list(range(E)))

    out = np.zeros((T, H), dtype=np.float32)
    for c in range(E):
        eA, ia, ca, eB, ib, cb = seg_tokens[c]
        # yT [128, NHT*caps] -> y[t, h]: y[t, i*128+p] = yT[p, i*caps + t]
        yTe = res.results[c]["yT"].reshape(128, NHT, caps)
        for idx, cw, base in ((ia, ca, 0), (ib, cb, capA)):
            n = len(idx)
            ye = yTe[:, :, base:base + n].transpose(2, 1, 0).reshape(n, H)
            out[idx] += ye * cw[:, None]
    return out.reshape(B, S, H)
